# revision 12
# baseline (speedup 1.0000x reference)
"""Trainium2 Bass kernel for DiceLoss (nn_DiceLoss_12326556140285).

Full (unsharded) contract: kernel(input, target, std) -> scalar np.ndarray.
Data-parallel over batch: 64 samples -> 8 cores x 8 samples. Inputs are
cast to bf16 on the host (halves HBM traffic; rel-err ~5e-4).

Math (per sample, z = (x - thr)/std, thr = 0.9*max(target)):
  s = sigmoid(z) = (1 + w)/2,  w = tanh(z/2)
  t = target > thr ;  H = x > thr ;  r = relu(w)
  x' = where(H == t, t, s)
    num = 2*sum(x' t) + 1e-5 = T + StH + Stw - Str + 1e-5
    den = sum(t) + sum(x') + 1e-5
        = 1.5*T + 0.5*(Sr + SH + Stw) - Str + 1e-5
    loss_b = 1 - num/den ;  output = mean_b loss_b

Engine split per core (DVE accumulate ops run at 1x, so all global sums
are recovered on the PE instead):
  - DVE: bf16 4x-mode tensor_scalar passes with NO accum: t-compare
    (written 127-blocked), H-compare, relu; plus a 2-level tensor_tensor
    max fold feeding a GpSimd free-dim reduce for the target max.
  - ACT: one tanh pass.
  - PE, per 127-wide tile: stationary = [t(127 cols) | ones] so PSUM row
    127 carries column sums (-> Sr, SH); moving = three dense regions
    [w | r | H] read as one 3D AP of overlapping 128-wide slices at
    stride 127 (garbage column 127 of each block is never extracted);
    plus a 1-column-ones-stationary matmul streaming the t tile into
    PSUM row 0 cols 384.. (-> T).  Diagonals (via eye with [127,127]=0)
    give Stw, Str, StH; a row-127 mask matrix gives Sr, SH.
"""

import numpy as np

N_CORES = 8
B = 64
SPC = B // N_CORES          # samples per core
DATA = 1024 * 1024 // 128   # 8192 data elems per partition per sample
TILE = 127                  # t columns per PE tile (col 128 is ones)
NT = 65                     # tiles per sample: 65*127 = 8255 >= 8192
R = 8256                    # region stride (even, >= NT*TILE)
N_ATOM = 8                  # T, SH, Sr, Stw, Str, StH, pad, pad

_COMPILED = {}


def build_nc(samples=SPC):
    import concourse.bass as bass
    import concourse.tile as tile
    from concourse import bacc, mybir, bass_isa

    f32 = mybir.dt.float32
    bf16 = mybir.dt.bfloat16
    Alu = mybir.AluOpType
    Act = mybir.ActivationFunctionType

    nc = bacc.Bacc("TRN2", target_bir_lowering=False, debug=False)
    inp_d = nc.dram_tensor("inp", [samples, 128, DATA], bf16, kind="ExternalInput").ap()
    tgt_d = nc.dram_tensor("tgt", [samples, 128, DATA], bf16, kind="ExternalInput").ap()
    std_d = nc.dram_tensor("std", [128, 1], f32, kind="ExternalInput").ap()
    eye_d = nc.dram_tensor("eye", [128, 128], f32, kind="ExternalInput").ap()
    msk_d = nc.dram_tensor("msk", [128, 128], f32, kind="ExternalInput").ap()
    out_d = nc.dram_tensor("out", [1, 1], f32, kind="ExternalOutput").ap()

    with tile.TileContext(nc) as tc:
        with (
            tc.tile_pool(name="const", bufs=1) as p_const,
            tc.tile_pool(name="tgt", bufs=2) as p_tgt,
            tc.tile_pool(name="x", bufs=2) as p_x,
            tc.tile_pool(name="tb", bufs=2) as p_t,
            tc.tile_pool(name="wrh", bufs=2) as p_wrh,
            tc.tile_pool(name="thr", bufs=2) as p_thr,
            tc.tile_pool(name="fin", bufs=16) as p_fin,
            tc.tile_pool(name="psum", bufs=2, space="PSUM") as p_psum,
        ):
            # ---- global constants ----
            eye = p_const.tile([128, 128], f32)      # eye, [127,127] zeroed
            nc.sync.dma_start(eye[:], eye_d[:])
            msk = p_const.tile([128, 128], f32)      # row 127 = 1 (j<127)
            nc.sync.dma_start(msk[:], msk_d[:])
            atoms = p_const.tile([128, samples * N_ATOM], f32)
            nc.vector.memset(atoms[:], 0.0)
            junk_f = p_const.tile([128, 128], f32)
            ones_st = p_const.tile([128, 1], bf16)   # tiny-matmul stationary
            nc.vector.memset(ones_st[:], 1.0)

            # 1/(2*std) and -1/(2*std) per partition (std replicated by host)
            std_sb = p_const.tile([128, 1], f32)
            nc.sync.dma_start(std_sb[:], std_d[:])
            std2 = p_const.tile([128, 1], f32)
            nc.vector.tensor_scalar_mul(std2[:], std_sb[:], 2.0)
            i2s = p_const.tile([128, 1], f32)
            nc.vector.reciprocal(i2s[:], std2[:])
            ni2s = p_const.tile([128, 1], f32)
            nc.vector.tensor_scalar_mul(ni2s[:], i2s[:], -1.0)

            for b in range(samples):
                ab = b * N_ATOM  # atom cols: T,SH,Sr,Stw,Str,StH

                # ---- target sample in SBUF (pad cols = -1 < thr) ----
                tgt_sb = p_tgt.tile([128, R], bf16)
                nc.vector.memset(tgt_sb[:, DATA:R], -1.0)
                for c in range(2):
                    sl = slice(c * DATA // 2, (c + 1) * DATA // 2)
                    nc.sync.dma_start(tgt_sb[:, sl], tgt_d[b][:, sl])

                # ---- wrh regions: w@0, r@R, H@2R ----
                wrh = p_wrh.tile([128, 3 * R], bf16)
                w_r = wrh[:, 0:R]
                r_r = wrh[:, R:2 * R]
                h_r = wrh[:, 2 * R:3 * R]

                # ---- max(target): 2 TT folds into H-region scratch,
                #      then GpSimd free-dim reduce ----
                nc.vector.tensor_tensor(
                    h_r[:, 0:4096], tgt_sb[:, 0:4096], tgt_sb[:, 4096:8192],
                    Alu.max,
                )
                nc.vector.tensor_tensor(
                    h_r[:, 4096:6144], h_r[:, 0:2048], h_r[:, 2048:4096],
                    Alu.max,
                )
                pmax = p_thr.tile([1, 1], f32)
                nc.gpsimd.reduce_max(
                    out=pmax[:], in_=h_r[:, 4096:6144],
                    axis=mybir.AxisListType.XYZWC,
                )
                allmax = p_thr.tile([128, 1], f32)
                nc.gpsimd.partition_broadcast(allmax[:], pmax[:])
                thr_t = p_thr.tile([128, 1], f32)
                nc.vector.tensor_scalar_mul(thr_t[:], allmax[:], 0.9)
                bias_t = p_thr.tile([128, 1], f32)  # -thr/(2 std)
                nc.vector.tensor_scalar(
                    bias_t[:], thr_t[:], ni2s[:], None, Alu.mult
                )

                # ---- input sample in SBUF (pad cols = -2 < thr) ----
                x_sb = p_x.tile([128, R], bf16)
                nc.vector.memset(x_sb[:, DATA:R], -2.0)
                for c in range(2):
                    sl = slice(c * DATA // 2, (c + 1) * DATA // 2)
                    nc.sync.dma_start(x_sb[:, sl], inp_d[b][:, sl])

                # ---- t tiles: [t(127)|1] interleaved (blocked write) ----
                t_big = p_t.tile([128, NT * 128], bf16)
                t_big3 = t_big[:].rearrange("p (t l) -> p t l", l=128)
                nc.vector.memset(t_big3[:, :, 127:128], 1.0)
                nc.vector.tensor_scalar(
                    t_big3[:, :, 0:127],
                    tgt_sb[:, 0:NT * TILE].rearrange("p (t l) -> p t l", l=TILE),
                    thr_t[:], None, Alu.is_gt,
                )

                # ---- ACT: w = tanh((x - thr)/(2 std)) ----
                nc.scalar.activation(
                    w_r, x_sb[:], Act.Tanh, bias=bias_t[:], scale=i2s[:],
                )
                # ---- DVE 4x passes: H = x > thr ; r = relu(w) ----
                nc.vector.tensor_scalar(
                    h_r, x_sb[:], thr_t[:], None, Alu.is_gt,
                )
                nc.vector.tensor_scalar(
                    r_r, w_r, 0.0, None, Alu.max,
                )

                # ---- PE: stationary [t|1], moving [w|r|H] 3D + T-matmul ----
                ps = p_psum.tile([128, 512], f32)
                wrh3 = wrh[:].rearrange("p (k f) -> p k f", k=3)
                for ti in range(NT):
                    nc.tensor.matmul(
                        ps[:, 0:384],
                        t_big[:, ti * 128:(ti + 1) * 128],
                        wrh3[:, :, ti * TILE: ti * TILE + 128],
                        start=(ti == 0), stop=False,
                    )
                    nc.tensor.matmul(
                        ps[0:1, 384:512],
                        ones_st[:],
                        t_big[:, ti * 128:(ti + 1) * 128],
                        start=False, stop=(ti == NT - 1),
                    )

                # ---- extract: diagonals -> Stw, Str, StH; row127 -> Sr, SH;
                #      T = sum of t column sums (psum row 0, cols 384..510) ----
                nc.vector.scalar_tensor_tensor(
                    junk_f[:], ps[:, 0:128], 1.0, eye[:], Alu.mult, Alu.mult,
                    accum_out=atoms[:, ab + 3: ab + 4],
                )
                nc.vector.scalar_tensor_tensor(
                    junk_f[:], ps[:, 128:256], 1.0, eye[:], Alu.mult, Alu.mult,
                    accum_out=atoms[:, ab + 4: ab + 5],
                )
                nc.vector.scalar_tensor_tensor(
                    junk_f[:], ps[:, 256:384], 1.0, eye[:], Alu.mult, Alu.mult,
                    accum_out=atoms[:, ab + 5: ab + 6],
                )
                nc.vector.scalar_tensor_tensor(
                    junk_f[:], ps[:, 128:256], 1.0, msk[:], Alu.mult, Alu.mult,
                    accum_out=atoms[:, ab + 2: ab + 3],
                )
                nc.vector.scalar_tensor_tensor(
                    junk_f[:], ps[:, 256:384], 1.0, msk[:], Alu.mult, Alu.mult,
                    accum_out=atoms[:, ab + 1: ab + 2],
                )
                nc.vector.reduce_sum(
                    out=atoms[0:1, ab: ab + 1], in_=ps[0:1, 384:511],
                    axis=mybir.AxisListType.X,
                )

            # ---- final reduction & loss assembly ----
            allat = p_fin.tile([128, samples * N_ATOM], f32)
            nc.gpsimd.partition_all_reduce(
                allat[:], atoms[:], channels=128,
                reduce_op=bass_isa.ReduceOp.add,
            )
            a = allat[0:1, :].rearrange("p (b k) -> p b k", k=N_ATOM)
            T, SH, Sr, Stw, Str, StH = (a[:, :, j] for j in range(6))

            _tvn = [0]

            def tv():
                _tvn[0] += 1
                return p_fin.tile(
                    [1, samples], f32, tag="fintmp", name=f"fintmp{_tvn[0]}"
                )

            # num = T + StH + Stw - Str + 1e-5
            # den = 1.5*T + 0.5*(Sr + SH + Stw) - Str + 1e-5
            n1 = tv(); nc.vector.tensor_add(n1[:], T, StH)
            n2 = tv(); nc.vector.tensor_sub(n2[:], Stw, Str)
            n3 = tv(); nc.vector.tensor_add(n3[:], n1[:], n2[:])
            num = tv(); nc.vector.tensor_scalar_add(num[:], n3[:], 1e-5)

            d1 = tv(); nc.vector.tensor_add(d1[:], Sr, SH)
            d2 = tv(); nc.vector.tensor_add(d2[:], d1[:], Stw)
            t15 = tv(); nc.vector.tensor_scalar_mul(t15[:], T, 1.5)
            d3 = tv(); nc.vector.scalar_tensor_tensor(
                d3[:], d2[:], 0.5, t15[:], Alu.mult, Alu.add
            )
            d4 = tv(); nc.vector.tensor_sub(d4[:], d3[:], Str)
            den = tv(); nc.vector.tensor_scalar_add(den[:], d4[:], 1e-5)

            rv = tv(); nc.vector.reciprocal(rv[:], den[:])
            pv = tv(); nc.vector.tensor_mul(pv[:], num[:], rv[:])
            sv = p_fin.tile([1, 1], f32, tag="finsc")
            nc.vector.reduce_sum(out=sv[:], in_=pv[:], axis=mybir.AxisListType.X)
            # sum_b (1 - pv_b) / B  (partial over this core's samples)
            outsb = p_fin.tile([1, 1], f32, tag="finout")
            nc.vector.tensor_scalar(
                outsb[:], sv[:], -1.0 / B, float(samples) / B, Alu.mult, Alu.add
            )
            nc.sync.dma_start(out_d[:], outsb[:])

    nc.compile()
    return nc


def _get_compiled():
    if "nc" not in _COMPILED:
        _COMPILED["nc"] = build_nc()
    return _COMPILED["nc"]


def _make_in_maps(input, target, std):
    import ml_dtypes

    bf16 = ml_dtypes.bfloat16
    inp = np.asarray(input, dtype=np.float32).reshape(B, 128, DATA).astype(bf16)
    tgt = np.asarray(target, dtype=np.float32).reshape(B, 128, DATA).astype(bf16)
    stdv = np.full((128, 1), np.asarray(std, dtype=np.float32).reshape(-1)[0],
                   dtype=np.float32)
    eye = np.eye(128, dtype=np.float32)
    eye[127, 127] = 0.0
    msk = np.zeros((128, 128), dtype=np.float32)
    msk[127, 0:127] = 1.0

    in_maps = []
    for c in range(N_CORES):
        sl = slice(c * SPC, (c + 1) * SPC)
        in_maps.append({
            "inp": np.ascontiguousarray(inp[sl]),
            "tgt": np.ascontiguousarray(tgt[sl]),
            "std": stdv,
            "eye": eye,
            "msk": msk,
        })
    return in_maps


def kernel(input, target, std):
    from concourse.bass_utils import run_bass_kernel_spmd

    nc = _get_compiled()
    in_maps = _make_in_maps(input, target, std)
    res = run_bass_kernel_spmd(nc, in_maps, list(range(N_CORES)))
    total = np.float32(0.0)
    for c in range(N_CORES):
        total += np.float32(res.results[c]["out"][0, 0])
    return np.array(total, dtype=np.float32)


# revision 15
# speedup vs baseline: 1.5235x; 1.5235x over previous
"""Trainium2 Bass kernel for DiceLoss (nn_DiceLoss_12326556140285).

Full (unsharded) contract: kernel(input, target, std) -> scalar np.ndarray.
Data-parallel over batch: 64 samples -> 8 cores x 8 samples. Inputs are
cast to bf16 on the host (halves HBM traffic; rel-err ~5e-4).

Math (per sample, z = (x - thr)/std, thr = 0.9*max(target)):
  s = sigmoid(z) = (1 + w)/2,  w = tanh(z/2)
  t = target > thr ;  H = x > thr ;  r = relu(w)
  x' = where(H == t, t, s)
    num = 2*sum(x' t) + 1e-5 = T + StH + Stw - Str + 1e-5
    den = sum(t) + sum(x') + 1e-5
        = 1.5*T + 0.5*(Sr + SH + Stw) - Str + 1e-5
    loss_b = 1 - num/den ;  output = mean_b loss_b

Engine split per core (DVE accumulate ops run at 1x on HW, so all sums
are recovered on the PE):
  - The free dim is tiled 127-wide (65 tiles/sample). A per-tile 512-col
    block [w|r|H|t] holds 128-wide overlapping windows (stride 127) of
    each quantity; DVE/ACT write the subblocks at full speed (inner dim
    128, step 1), reading x/target through overlapping 127-stride views.
  - The t-subblock itself is the matmul stationary, with its column 127
    (the overlap duplicate) memset to 1.0: psum = [t|1]^T [w|r|H|t].
    Diagonals via eye(127) give Stw, Str, StH and T (t^T t); psum row
    127 (the ones row) gives the global sums Sr, SH via a row-mask
    matrix. Elements are counted exactly once (col 127 never extracted).
  - Target max: two tensor_tensor max folds (2x) into w-area scratch,
    then one GpSimd XYZWC reduce + partition_broadcast.
  - ACT: one tanh pass. Pad columns (x=-2, target=-1, below any thr)
    make all pad contributions exactly zero.
"""

import numpy as np

N_CORES = 8
B = 64
SPC = B // N_CORES          # samples per core
DATA = 1024 * 1024 // 128   # 8192 data elems per partition per sample
TILE = 127                  # elements advanced per PE tile
NT = 65                     # tiles per sample: 65*127 = 8255 >= 8192
R = 8256                    # padded dense sample width (even)
N_ATOM = 8                  # T, SH, Sr, Stw, Str, StH, pad, pad

_COMPILED = {}


def build_nc(samples=SPC):
    import concourse.bass as bass
    import concourse.tile as tile
    from concourse import bacc, mybir, bass_isa

    f32 = mybir.dt.float32
    bf16 = mybir.dt.bfloat16
    Alu = mybir.AluOpType
    Act = mybir.ActivationFunctionType

    nc = bacc.Bacc("TRN2", target_bir_lowering=False, debug=False)
    inp_d = nc.dram_tensor("inp", [samples, 128, DATA], bf16, kind="ExternalInput").ap()
    tgt_d = nc.dram_tensor("tgt", [samples, 128, DATA], bf16, kind="ExternalInput").ap()
    std_d = nc.dram_tensor("std", [128, 1], f32, kind="ExternalInput").ap()
    eye_d = nc.dram_tensor("eye", [128, 128], f32, kind="ExternalInput").ap()
    msk_d = nc.dram_tensor("msk", [128, 128], f32, kind="ExternalInput").ap()
    out_d = nc.dram_tensor("out", [1, 1], f32, kind="ExternalOutput").ap()

    with tile.TileContext(nc) as tc:
        with (
            tc.tile_pool(name="const", bufs=1) as p_const,
            tc.tile_pool(name="tgt", bufs=2) as p_tgt,
            tc.tile_pool(name="x", bufs=2) as p_x,
            tc.tile_pool(name="wrht", bufs=2) as p_wrht,
            tc.tile_pool(name="thr", bufs=2) as p_thr,
            tc.tile_pool(name="fin", bufs=16) as p_fin,
            tc.tile_pool(name="psum", bufs=2, space="PSUM") as p_psum,
        ):
            # ---- global constants ----
            eye = p_const.tile([128, 128], f32)      # eye, [127,127] zeroed
            nc.sync.dma_start(eye[:], eye_d[:])
            msk = p_const.tile([128, 128], f32)      # row 127 = 1 (j<127)
            nc.sync.dma_start(msk[:], msk_d[:])
            atoms = p_const.tile([128, samples * N_ATOM], f32)
            nc.vector.memset(atoms[:], 0.0)
            junk_f = p_const.tile([128, 128], f32)

            # 1/(2*std) and -1/(2*std) per partition (std replicated by host)
            std_sb = p_const.tile([128, 1], f32)
            nc.sync.dma_start(std_sb[:], std_d[:])
            std2 = p_const.tile([128, 1], f32)
            nc.vector.tensor_scalar_mul(std2[:], std_sb[:], 2.0)
            i2s = p_const.tile([128, 1], f32)
            nc.vector.reciprocal(i2s[:], std2[:])
            ni2s = p_const.tile([128, 1], f32)
            nc.vector.tensor_scalar_mul(ni2s[:], i2s[:], -1.0)

            for b in range(samples):
                ab = b * N_ATOM  # atom cols: T,SH,Sr,Stw,Str,StH

                # ---- target sample in SBUF (pad cols = -1 < thr) ----
                tgt_sb = p_tgt.tile([128, R], bf16)
                nc.vector.memset(tgt_sb[:, DATA:R], -1.0)
                for c in range(2):
                    sl = slice(c * DATA // 2, (c + 1) * DATA // 2)
                    nc.sync.dma_start(tgt_sb[:, sl], tgt_d[b][:, sl])

                # per-tile [w|r|H|t] blocks; subblock k of tile ti holds
                # 128-wide window of quantity k at element base ti*127
                wrht = p_wrht.tile([128, NT * 512], bf16)
                wb = wrht[:].rearrange("p (t k l) -> p t k l", k=4, l=128)
                w_v = wb[:, :, 0, :]
                r_v = wb[:, :, 1, :]
                h_v = wb[:, :, 2, :]
                t_v = wb[:, :, 3, :]

                # ---- max(target): 2 TT folds into w-area scratch,
                #      then GpSimd XYZWC reduce + broadcast ----
                nc.vector.tensor_tensor(
                    wrht[:, 0:4096], tgt_sb[:, 0:4096], tgt_sb[:, 4096:8192],
                    Alu.max,
                )
                nc.vector.tensor_tensor(
                    wrht[:, 4096:6144], wrht[:, 0:2048], wrht[:, 2048:4096],
                    Alu.max,
                )
                pmax = p_thr.tile([1, 1], f32)
                nc.gpsimd.reduce_max(
                    out=pmax[:], in_=wrht[:, 4096:6144],
                    axis=mybir.AxisListType.XYZWC,
                )
                allmax = p_thr.tile([128, 1], f32)
                nc.gpsimd.partition_broadcast(allmax[:], pmax[:])
                thr_t = p_thr.tile([128, 1], f32)
                nc.vector.tensor_scalar_mul(thr_t[:], allmax[:], 0.9)
                bias_t = p_thr.tile([128, 1], f32)  # -thr/(2 std)
                nc.vector.tensor_scalar(
                    bias_t[:], thr_t[:], ni2s[:], None, Alu.mult
                )

                # ---- input sample in SBUF (pad cols = -2 < thr) ----
                x_sb = p_x.tile([128, R], bf16)
                nc.vector.memset(x_sb[:, DATA:R], -2.0)
                for c in range(2):
                    sl = slice(c * DATA // 2, (c + 1) * DATA // 2)
                    nc.sync.dma_start(x_sb[:, sl], inp_d[b][:, sl])

                # t = target > thr  (blocked out, overlap in)
                nc.vector.tensor_scalar(
                    t_v, _ovl(tgt_sb[:], TILE, NT, 128), thr_t[:], None,
                    Alu.is_gt,
                )
                # ones column: overwrite t col 127 of every tile
                nc.vector.memset(wb[:, :, 3, 127:128], 1.0)

                # ---- ACT: w = tanh((x - thr)/(2 std)) ----
                nc.scalar.activation(
                    w_v, _ovl(x_sb[:], TILE, NT, 128), Act.Tanh,
                    bias=bias_t[:], scale=i2s[:],
                )
                # ---- DVE 4x passes: H = x > thr ; r = relu(w) ----
                nc.vector.tensor_scalar(
                    h_v, _ovl(x_sb[:], TILE, NT, 128), thr_t[:], None,
                    Alu.is_gt,
                )
                nc.vector.tensor_scalar(
                    r_v, w_v, 0.0, None, Alu.max,
                )

                # ---- PE: stationary = t-subblock ([t(127)|1]),
                #      moving = the whole [w|r|H|t] tile (contiguous) ----
                ps = p_psum.tile([128, 512], f32)
                for ti in range(NT):
                    nc.tensor.matmul(
                        ps[:],
                        wb[:, ti, 3, :],
                        wrht[:, ti * 512:(ti + 1) * 512],
                        start=(ti == 0), stop=(ti == NT - 1),
                    )

                # ---- extract: diagonals -> Stw, Str, StH, T;
                #      ones row 127 -> Sr, SH ----
                for col, reg in ((ab + 3, 0), (ab + 4, 1), (ab + 5, 2), (ab + 0, 3)):
                    nc.vector.scalar_tensor_tensor(
                        junk_f[:], ps[:, reg * 128:(reg + 1) * 128], 1.0,
                        eye[:], Alu.mult, Alu.mult,
                        accum_out=atoms[:, col: col + 1],
                    )
                for col, reg in ((ab + 2, 1), (ab + 1, 2)):
                    nc.vector.scalar_tensor_tensor(
                        junk_f[:], ps[:, reg * 128:(reg + 1) * 128], 1.0,
                        msk[:], Alu.mult, Alu.mult,
                        accum_out=atoms[:, col: col + 1],
                    )

            # ---- final reduction & loss assembly ----
            allat = p_fin.tile([128, samples * N_ATOM], f32)
            nc.gpsimd.partition_all_reduce(
                allat[:], atoms[:], channels=128,
                reduce_op=bass_isa.ReduceOp.add,
            )
            a = allat[0:1, :].rearrange("p (b k) -> p b k", k=N_ATOM)
            T, SH, Sr, Stw, Str, StH = (a[:, :, j] for j in range(6))

            _tvn = [0]

            def tv():
                _tvn[0] += 1
                return p_fin.tile(
                    [1, samples], f32, tag="fintmp", name=f"fintmp{_tvn[0]}"
                )

            # num = T + StH + Stw - Str + 1e-5
            # den = 1.5*T + 0.5*(Sr + SH + Stw) - Str + 1e-5
            n1 = tv(); nc.vector.tensor_add(n1[:], T, StH)
            n2 = tv(); nc.vector.tensor_sub(n2[:], Stw, Str)
            n3 = tv(); nc.vector.tensor_add(n3[:], n1[:], n2[:])
            num = tv(); nc.vector.tensor_scalar_add(num[:], n3[:], 1e-5)

            d1 = tv(); nc.vector.tensor_add(d1[:], Sr, SH)
            d2 = tv(); nc.vector.tensor_add(d2[:], d1[:], Stw)
            t15 = tv(); nc.vector.tensor_scalar_mul(t15[:], T, 1.5)
            d3 = tv(); nc.vector.scalar_tensor_tensor(
                d3[:], d2[:], 0.5, t15[:], Alu.mult, Alu.add
            )
            d4 = tv(); nc.vector.tensor_sub(d4[:], d3[:], Str)
            den = tv(); nc.vector.tensor_scalar_add(den[:], d4[:], 1e-5)

            rv = tv(); nc.vector.reciprocal(rv[:], den[:])
            pv = tv(); nc.vector.tensor_mul(pv[:], num[:], rv[:])
            sv = p_fin.tile([1, 1], f32, tag="finsc")
            nc.vector.reduce_sum(out=sv[:], in_=pv[:], axis=mybir.AxisListType.X)
            # sum_b (1 - pv_b) / B  (partial over this core's samples)
            outsb = p_fin.tile([1, 1], f32, tag="finout")
            nc.vector.tensor_scalar(
                outsb[:], sv[:], -1.0 / B, float(samples) / B, Alu.mult, Alu.add
            )
            nc.sync.dma_start(out_d[:], outsb[:])

    nc.compile()
    return nc


def _ovl(ap, stride, n, width):
    """[128, F] -> [128, n, width] overlapping windows at the given stride."""
    v = ap.copy()
    lst = v.ap
    lst[1:] = [[stride, n], [1, width]]
    v.ap = lst
    return v


def _get_compiled():
    if "nc" not in _COMPILED:
        _COMPILED["nc"] = build_nc()
    return _COMPILED["nc"]


def _make_in_maps(input, target, std):
    import ml_dtypes

    bf16 = ml_dtypes.bfloat16
    inp = np.asarray(input, dtype=np.float32).reshape(B, 128, DATA).astype(bf16)
    tgt = np.asarray(target, dtype=np.float32).reshape(B, 128, DATA).astype(bf16)
    stdv = np.full((128, 1), np.asarray(std, dtype=np.float32).reshape(-1)[0],
                   dtype=np.float32)
    eye = np.eye(128, dtype=np.float32)
    eye[127, 127] = 0.0
    msk = np.zeros((128, 128), dtype=np.float32)
    msk[127, 0:127] = 1.0

    in_maps = []
    for c in range(N_CORES):
        sl = slice(c * SPC, (c + 1) * SPC)
        in_maps.append({
            "inp": np.ascontiguousarray(inp[sl]),
            "tgt": np.ascontiguousarray(tgt[sl]),
            "std": stdv,
            "eye": eye,
            "msk": msk,
        })
    return in_maps


def kernel(input, target, std):
    from concourse.bass_utils import run_bass_kernel_spmd

    nc = _get_compiled()
    in_maps = _make_in_maps(input, target, std)
    res = run_bass_kernel_spmd(nc, in_maps, list(range(N_CORES)))
    total = np.float32(0.0)
    for c in range(N_CORES):
        total += np.float32(res.results[c]["out"][0, 0])
    return np.array(total, dtype=np.float32)


# revision 17
# speedup vs baseline: 1.6334x; 1.0721x over previous
"""Trainium2 Bass kernel for DiceLoss (nn_DiceLoss_12326556140285).

Full (unsharded) contract: kernel(input, target, std) -> scalar np.ndarray.
Data-parallel over batch: 64 samples -> 8 cores x 8 samples. Inputs are
cast to bf16 on the host (halves HBM traffic; rel-err ~5e-4).

Math (per sample, z = (x - thr)/std, thr = 0.9*max(target)):
  s = sigmoid(z) = (1 + w)/2,  w = tanh(z/2)
  t = target > thr ;  H = x > thr ;  r = relu(w)
  x' = where(H == t, t, s)
    num = 2*sum(x' t) + 1e-5 = T + StH + Stw - Str + 1e-5
    den = sum(t) + sum(x') + 1e-5
        = 1.5*T + 0.5*(Sr + SH + Stw) - Str + 1e-5
    loss_b = 1 - num/den ;  output = mean_b loss_b

Engine split per core (DVE accumulate ops run at 1x on HW, so all sums
are recovered on the PE):
  - The free dim is tiled 127-wide (65 tiles/sample). A per-tile 512-col
    block [w|r|H|t] holds 128-wide overlapping windows (stride 127) of
    each quantity; DVE/ACT write the subblocks at full speed (inner dim
    128, step 1), reading x/target through overlapping 127-stride views.
  - The t-subblock itself is the matmul stationary, with its column 127
    (the overlap duplicate) memset to 1.0: psum = [t|1]^T [w|r|H|t].
    Diagonals via eye(127) give Stw, Str, StH and T (t^T t); psum row
    127 (the ones row) gives the global sums Sr, SH via a row-mask
    matrix. Elements are counted exactly once (col 127 never extracted).
  - Target max: two tensor_tensor max folds (2x) into w-area scratch,
    then one GpSimd XYZWC reduce + partition_broadcast.
  - ACT: one tanh pass. Pad columns (x=-2, target=-1, below any thr)
    make all pad contributions exactly zero.
"""

import numpy as np

N_CORES = 8
B = 64
SPC = B // N_CORES          # samples per core
DATA = 1024 * 1024 // 128   # 8192 data elems per partition per sample
TILE = 127                  # elements advanced per PE tile
NT = 65                     # tiles per sample: 65*127 = 8255 >= 8192
R = 8256                    # padded dense sample width (even)
N_ATOM = 8                  # T, SH, Sr, Stw, Str, StH, pad, pad

_COMPILED = {}


def build_nc(samples=SPC):
    import concourse.bass as bass
    import concourse.tile as tile
    from concourse import bacc, mybir, bass_isa

    f32 = mybir.dt.float32
    bf16 = mybir.dt.bfloat16
    Alu = mybir.AluOpType
    Act = mybir.ActivationFunctionType

    nc = bacc.Bacc("TRN2", target_bir_lowering=False, debug=False)
    inp_d = nc.dram_tensor("inp", [samples, 128, DATA], bf16, kind="ExternalInput").ap()
    tgt_d = nc.dram_tensor("tgt", [samples, 128, DATA], bf16, kind="ExternalInput").ap()
    std_d = nc.dram_tensor("std", [128, 1], f32, kind="ExternalInput").ap()
    eye_d = nc.dram_tensor("eye", [128, 128], f32, kind="ExternalInput").ap()
    msk_d = nc.dram_tensor("msk", [128, 128], f32, kind="ExternalInput").ap()
    out_d = nc.dram_tensor("out", [1, 1], f32, kind="ExternalOutput").ap()

    with tile.TileContext(nc) as tc:
        with (
            tc.tile_pool(name="const", bufs=1) as p_const,
            tc.tile_pool(name="tgt", bufs=2) as p_tgt,
            tc.tile_pool(name="x", bufs=2) as p_x,
            tc.tile_pool(name="wrht", bufs=2) as p_wrht,
            tc.tile_pool(name="thr", bufs=2) as p_thr,
            tc.tile_pool(name="fin", bufs=16) as p_fin,
            tc.tile_pool(name="psum", bufs=2, space="PSUM") as p_psum,
        ):
            # ---- global constants ----
            eye = p_const.tile([128, 128], f32)      # eye, [127,127] zeroed
            nc.sync.dma_start(eye[:], eye_d[:])
            msk = p_const.tile([128, 128], f32)      # row 127 = 1 (j<127)
            nc.sync.dma_start(msk[:], msk_d[:])
            atoms = p_const.tile([128, samples * N_ATOM], f32)
            nc.vector.memset(atoms[:], 0.0)
            junk_f = p_const.tile([128, 128], f32)

            # 1/(2*std) and -1/(2*std) per partition (std replicated by host)
            std_sb = p_const.tile([128, 1], f32)
            nc.sync.dma_start(std_sb[:], std_d[:])
            std2 = p_const.tile([128, 1], f32)
            nc.vector.tensor_scalar_mul(std2[:], std_sb[:], 2.0)
            i2s = p_const.tile([128, 1], f32)
            nc.vector.reciprocal(i2s[:], std2[:])
            ni2s = p_const.tile([128, 1], f32)
            nc.vector.tensor_scalar_mul(ni2s[:], i2s[:], -1.0)

            for b in range(samples):
                ab = b * N_ATOM  # atom cols: T,SH,Sr,Stw,Str,StH

                # ---- target sample in SBUF (pad cols = -1 < thr) ----
                tgt_sb = p_tgt.tile([128, R], bf16)
                nc.vector.memset(tgt_sb[:, DATA:R], -1.0)
                for c in range(2):
                    sl = slice(c * DATA // 2, (c + 1) * DATA // 2)
                    nc.sync.dma_start(tgt_sb[:, sl], tgt_d[b][:, sl])

                # per-tile [w|r|H|t] blocks; subblock k of tile ti holds
                # 128-wide window of quantity k at element base ti*127
                wrht = p_wrht.tile([128, NT * 512], bf16)
                wb = wrht[:].rearrange("p (t k l) -> p t k l", k=4, l=128)
                w_v = wb[:, :, 0, :]
                r_v = wb[:, :, 1, :]
                h_v = wb[:, :, 2, :]
                t_v = wb[:, :, 3, :]

                # ---- max(target): 2 TT folds into w-area scratch,
                #      then GpSimd XYZWC reduce + broadcast ----
                nc.vector.tensor_tensor(
                    wrht[:, 0:4096], tgt_sb[:, 0:4096], tgt_sb[:, 4096:8192],
                    Alu.max,
                )
                nc.vector.tensor_tensor(
                    wrht[:, 4096:6144], wrht[:, 0:2048], wrht[:, 2048:4096],
                    Alu.max,
                )
                pmax = p_thr.tile([1, 1], f32)
                nc.gpsimd.reduce_max(
                    out=pmax[:], in_=wrht[:, 4096:6144],
                    axis=mybir.AxisListType.XYZWC,
                )
                allmax = p_thr.tile([128, 1], f32)
                nc.gpsimd.partition_broadcast(allmax[:], pmax[:])
                thr_t = p_thr.tile([128, 1], f32)
                nc.vector.tensor_scalar_mul(thr_t[:], allmax[:], 0.9)
                bias_t = p_thr.tile([128, 1], f32)  # -thr/(2 std)
                nc.vector.tensor_scalar(
                    bias_t[:], thr_t[:], ni2s[:], None, Alu.mult
                )

                # ---- input sample in SBUF (pad cols = -2 < thr) ----
                x_sb = p_x.tile([128, R], bf16)
                nc.vector.memset(x_sb[:, DATA:R], -2.0)
                for c in range(2):
                    sl = slice(c * DATA // 2, (c + 1) * DATA // 2)
                    nc.sync.dma_start(x_sb[:, sl], inp_d[b][:, sl])

                # ---- chunked passes + matmuls so the PE stream starts
                #      early and never idles long enough to re-throttle ----
                ps = p_psum.tile([128, 512], f32)
                bounds = [0, 17, 33, 49, NT]
                for c in range(4):
                    lo, hi = bounds[c], bounds[c + 1]
                    n = hi - lo
                    x_ov = _ovl(x_sb[:, lo * TILE:], TILE, n, 128)
                    tg_ov = _ovl(tgt_sb[:, lo * TILE:], TILE, n, 128)
                    # t = target > thr (blocked out, overlap in), then the
                    # ones column overwrites t col 127 of each tile
                    nc.vector.tensor_scalar(
                        t_v[:, lo:hi, :], tg_ov, thr_t[:], None, Alu.is_gt,
                    )
                    nc.vector.memset(wb[:, lo:hi, 3, 127:128], 1.0)
                    # w = tanh((x - thr)/(2 std))
                    nc.scalar.activation(
                        w_v[:, lo:hi, :], x_ov, Act.Tanh,
                        bias=bias_t[:], scale=i2s[:],
                    )
                    # H = x > thr ; r = relu(w)
                    nc.vector.tensor_scalar(
                        h_v[:, lo:hi, :], x_ov, thr_t[:], None, Alu.is_gt,
                    )
                    nc.vector.tensor_scalar(
                        r_v[:, lo:hi, :], w_v[:, lo:hi, :], 0.0, None, Alu.max,
                    )
                    # PE: stationary = t-subblock ([t(127)|1]),
                    # moving = the whole [w|r|H|t] tile (contiguous)
                    for ti in range(lo, hi):
                        nc.tensor.matmul(
                            ps[:],
                            wb[:, ti, 3, :],
                            wrht[:, ti * 512:(ti + 1) * 512],
                            start=(ti == 0), stop=(ti == NT - 1),
                        )

                # ---- extract: diagonals -> Stw, Str, StH, T;
                #      ones row 127 -> Sr, SH ----
                for col, reg in ((ab + 3, 0), (ab + 4, 1), (ab + 5, 2), (ab + 0, 3)):
                    nc.vector.scalar_tensor_tensor(
                        junk_f[:], ps[:, reg * 128:(reg + 1) * 128], 1.0,
                        eye[:], Alu.mult, Alu.mult,
                        accum_out=atoms[:, col: col + 1],
                    )
                for col, reg in ((ab + 2, 1), (ab + 1, 2)):
                    nc.vector.scalar_tensor_tensor(
                        junk_f[:], ps[:, reg * 128:(reg + 1) * 128], 1.0,
                        msk[:], Alu.mult, Alu.mult,
                        accum_out=atoms[:, col: col + 1],
                    )

            # ---- final reduction & loss assembly ----
            allat = p_fin.tile([128, samples * N_ATOM], f32)
            nc.gpsimd.partition_all_reduce(
                allat[:], atoms[:], channels=128,
                reduce_op=bass_isa.ReduceOp.add,
            )
            a = allat[0:1, :].rearrange("p (b k) -> p b k", k=N_ATOM)
            T, SH, Sr, Stw, Str, StH = (a[:, :, j] for j in range(6))

            _tvn = [0]

            def tv():
                _tvn[0] += 1
                return p_fin.tile(
                    [1, samples], f32, tag="fintmp", name=f"fintmp{_tvn[0]}"
                )

            # num = T + StH + Stw - Str + 1e-5
            # den = 1.5*T + 0.5*(Sr + SH + Stw) - Str + 1e-5
            n1 = tv(); nc.vector.tensor_add(n1[:], T, StH)
            n2 = tv(); nc.vector.tensor_sub(n2[:], Stw, Str)
            n3 = tv(); nc.vector.tensor_add(n3[:], n1[:], n2[:])
            num = tv(); nc.vector.tensor_scalar_add(num[:], n3[:], 1e-5)

            d1 = tv(); nc.vector.tensor_add(d1[:], Sr, SH)
            d2 = tv(); nc.vector.tensor_add(d2[:], d1[:], Stw)
            t15 = tv(); nc.vector.tensor_scalar_mul(t15[:], T, 1.5)
            d3 = tv(); nc.vector.scalar_tensor_tensor(
                d3[:], d2[:], 0.5, t15[:], Alu.mult, Alu.add
            )
            d4 = tv(); nc.vector.tensor_sub(d4[:], d3[:], Str)
            den = tv(); nc.vector.tensor_scalar_add(den[:], d4[:], 1e-5)

            rv = tv(); nc.vector.reciprocal(rv[:], den[:])
            pv = tv(); nc.vector.tensor_mul(pv[:], num[:], rv[:])
            sv = p_fin.tile([1, 1], f32, tag="finsc")
            nc.vector.reduce_sum(out=sv[:], in_=pv[:], axis=mybir.AxisListType.X)
            # sum_b (1 - pv_b) / B  (partial over this core's samples)
            outsb = p_fin.tile([1, 1], f32, tag="finout")
            nc.vector.tensor_scalar(
                outsb[:], sv[:], -1.0 / B, float(samples) / B, Alu.mult, Alu.add
            )
            nc.sync.dma_start(out_d[:], outsb[:])

    nc.compile()
    return nc


def _ovl(ap, stride, n, width):
    """[128, F] -> [128, n, width] overlapping windows at the given stride."""
    v = ap.copy()
    lst = v.ap
    lst[1:] = [[stride, n], [1, width]]
    v.ap = lst
    return v


def _get_compiled():
    if "nc" not in _COMPILED:
        _COMPILED["nc"] = build_nc()
    return _COMPILED["nc"]


def _make_in_maps(input, target, std):
    import ml_dtypes

    bf16 = ml_dtypes.bfloat16
    inp = np.asarray(input, dtype=np.float32).reshape(B, 128, DATA).astype(bf16)
    tgt = np.asarray(target, dtype=np.float32).reshape(B, 128, DATA).astype(bf16)
    stdv = np.full((128, 1), np.asarray(std, dtype=np.float32).reshape(-1)[0],
                   dtype=np.float32)
    eye = np.eye(128, dtype=np.float32)
    eye[127, 127] = 0.0
    msk = np.zeros((128, 128), dtype=np.float32)
    msk[127, 0:127] = 1.0

    in_maps = []
    for c in range(N_CORES):
        sl = slice(c * SPC, (c + 1) * SPC)
        in_maps.append({
            "inp": np.ascontiguousarray(inp[sl]),
            "tgt": np.ascontiguousarray(tgt[sl]),
            "std": stdv,
            "eye": eye,
            "msk": msk,
        })
    return in_maps


def kernel(input, target, std):
    from concourse.bass_utils import run_bass_kernel_spmd

    nc = _get_compiled()
    in_maps = _make_in_maps(input, target, std)
    res = run_bass_kernel_spmd(nc, in_maps, list(range(N_CORES)))
    total = np.float32(0.0)
    for c in range(N_CORES):
        total += np.float32(res.results[c]["out"][0, 0])
    return np.array(total, dtype=np.float32)
